# revision 19
# baseline (speedup 1.0000x reference)
"""Trainium2 Bass kernel for a 4-layer CLUTRR transformer encoder + pooling head.

Strategy: data-parallel over batch (8 examples per core x 8 cores).
- Examples are sorted by valid length and dealt snake-wise so every core's
  slot j has a similar kv length; the program is specialized per slot to
  skip attention work beyond ceil(maxlen/128) key tiles.
- Residual stream kept in SBUF fp32 token-major; per-layer bf16 copies +
  DMA-xbar transposes provide the H-major operands for matmuls.
- Attention computed in "scoresT" layout [k_part, q_free]: key-side mask is a
  per-partition bias on the Exp activation; softmax denominator comes from a
  col-tiled ones-matmul; normalization folded into the PSUM->SBUF copy.
- All big matmuls run in bf16 (fp32 PSUM accumulation).
"""

import sys

sys.path.insert(0, "/opt/trn_rl_repo")

import numpy as np
import ml_dtypes

import concourse.bass as bass
import concourse.tile as tile
from concourse import bacc, mybir
from concourse.bass import IndirectOffsetOnAxis
from concourse.bass_utils import run_bass_kernel_spmd

F32 = mybir.dt.float32
BF16 = mybir.dt.bfloat16
I32 = mybir.dt.int32
AF = mybir.ActivationFunctionType
OP = mybir.AluOpType

B, S, H, L, NH, FF, V, KIDX, R = 64, 512, 256, 4, 8, 1024, 50265, 8, 25
DH = H // NH
EPS = 1e-5
NCORES = 8
EPC = B // NCORES  # examples per core
TPC = EPC * S // 128  # token tiles per core (32)
P = 128


def _bf(x):
    return np.asarray(x, dtype=np.float32).astype(ml_dtypes.bfloat16)


_PROG_CACHE = {}


def build_program_cached(KVT, QT, nz, reps=1, parts=None):
    key = (tuple(KVT), tuple(QT), tuple(sorted(nz.items())), reps,
           tuple(sorted(parts)) if parts else None)
    if key not in _PROG_CACHE:
        _PROG_CACHE[key] = build_program(KVT, QT, nz, reps=reps, parts=parts)
    return _PROG_CACHE[key]


def build_program(KVT, QT, nz, reps=1, parts=None):
    if parts is None:
        parts = {"gather", "trans", "attn", "wo", "ffn", "head"}
    """Build the per-core Bass program.

    KVT: list of EPC ints - number of valid 128-key tiles per example slot.
    QT: list of EPC ints - number of active (compacted) token tiles per slot.
    nz: dict of flags for which optional params are nonzero/non-identity.
    """
    TOT = sum(QT)                      # total compacted token tiles per core
    off = [sum(QT[:e]) for e in range(EPC)]
    assert all(KVT[e] <= QT[e] for e in range(EPC))

    nc = bacc.Bacc("TRN2", target_bir_lowering=False, debug=False,
                   num_devices=NCORES)

    # ---------------- DRAM tensors ----------------
    emb_d = nc.dram_tensor("emb", [V, H], F32, kind="ExternalInput")
    ids_d = nc.dram_tensor("ids", [P, TOT], I32, kind="ExternalInput")
    kbias_d = nc.dram_tensor("kbias", [P, EPC, 4], F32, kind="ExternalInput")
    wpool_d = nc.dram_tensor("wpool", [P, EPC, 4, 3], F32, kind="ExternalInput")
    wqkv_d = nc.dram_tensor("wqkv", [P, L, 2, 3 * H], BF16, kind="ExternalInput")
    wo_d = nc.dram_tensor("wo", [P, L, 2, H], BF16, kind="ExternalInput")
    w1_d = nc.dram_tensor("w1", [P, L, 2, FF], BF16, kind="ExternalInput")
    w2_d = nc.dram_tensor("w2", [P, L, 8, H], BF16, kind="ExternalInput")
    mw1_d = nc.dram_tensor("mw1", [P, 6, H], F32, kind="ExternalInput")
    mw2_d = nc.dram_tensor("mw2", [P, 2, R], F32, kind="ExternalInput")
    out_d = nc.dram_tensor("out", [EPC, R], F32, kind="ExternalOutput")

    # optional (nonzero) parameter tensors
    bqk_d = bqv_d = b1_d = bo_d = b2_d = mb1_d = mb2_d = lng_d = None
    if nz["bqk"]:
        bqk_d = nc.dram_tensor("bqk", [P, L, 4], F32, kind="ExternalInput")
    if nz["bqv"]:
        bqv_d = nc.dram_tensor("bqv", [L, H], BF16, kind="ExternalInput")
    if nz["b1"]:
        b1_d = nc.dram_tensor("b1", [P, L, 8], F32, kind="ExternalInput")
    if nz["bo"]:
        bo_d = nc.dram_tensor("bo", [L, H], BF16, kind="ExternalInput")
    if nz["b2"]:
        b2_d = nc.dram_tensor("b2", [L, H], BF16, kind="ExternalInput")
    if nz["mb1"]:
        mb1_d = nc.dram_tensor("mb1", [P, 2], F32, kind="ExternalInput")
    if nz["mb2"]:
        mb2_d = nc.dram_tensor("mb2", [1, R], F32, kind="ExternalInput")
    if nz["ln"]:
        # 9 rows: ln1_g,ln1_b,ln2_g,ln2_b per layer packed [4L+2, H] on host
        lng_d = nc.dram_tensor("lngb", [4 * L + 2, H], F32, kind="ExternalInput")

    from contextlib import ExitStack, nullcontext
    with tile.TileContext(nc) as tc, ExitStack() as ctx:
        pp = ctx.enter_context(tc.tile_pool(name="persist", bufs=1))
        wp = ctx.enter_context(tc.tile_pool(name="weights", bufs=2))
        sb = ctx.enter_context(tc.tile_pool(name="work", bufs=3))
        ps = ctx.enter_context(tc.tile_pool(name="psum", bufs=1, space="PSUM"))
        rep_cm = tc.For_i(0, reps, 1) if reps > 1 else nullcontext()
        ctx.enter_context(rep_cm)

        # ------------- persistent tiles -------------
        x_all = pp.tile([P, TOT, H], F32)     # residual stream, token-major
        x_bf = pp.tile([P, 2, TOT, P], BF16)  # bf16 cast, [tok, chunk, tile, h]
        xT = pp.tile([P, 2, TOT, P], BF16)    # H-major bf16 [h, chunk, tile, tok]
        oT_all = pp.tile([P, 2, EPC, S], BF16)  # attn out, H-major per example
        ids_sb = pp.tile([P, TOT], I32)
        kbias_sb = pp.tile([P, EPC, 4], F32)
        wpool_sb = pp.tile([P, EPC, 4, 3], F32)
        ones32 = pp.tile([P, 32], BF16)
        eps_col = pp.tile([P, 1], F32)
        mw1_sb = pp.tile([P, 6, H], F32)
        mw2_sb = pp.tile([P, 2, R], F32)
        featT = pp.tile([P, 3, 2, EPC], F32)  # [h, j(sub/obj/story), hc, e]

        nc.sync.dma_start(out=ids_sb, in_=ids_d[:, :])
        nc.sync.dma_start(out=kbias_sb, in_=kbias_d[:, :, :])
        nc.sync.dma_start(out=wpool_sb, in_=wpool_d[:, :, :, :])
        nc.sync.dma_start(out=mw1_sb, in_=mw1_d[:, :, :])
        nc.sync.dma_start(out=mw2_sb, in_=mw2_d[:, :, :])
        nc.vector.memset(ones32, 1.0)
        nc.vector.memset(eps_col, EPS)

        bqk_sb = bqv_sb = b1_sb = bo_sb = b2_sb = mb1_sb = mb2_sb = None
        ones_row = None  # [1,128] bf16 for K=1 bias matmuls
        if nz["bqk"]:
            bqk_sb = pp.tile([P, L, 4], F32)
            nc.sync.dma_start(out=bqk_sb, in_=bqk_d[:, :, :])
        if nz["b1"]:
            b1_sb = pp.tile([P, L, 8], F32)
            nc.sync.dma_start(out=b1_sb, in_=b1_d[:, :, :])
        if nz["mb1"]:
            mb1_sb = pp.tile([P, 2], F32)
            nc.sync.dma_start(out=mb1_sb, in_=mb1_d[:, :])
        if nz["bqv"] or nz["bo"] or nz["b2"]:
            ones_row = pp.tile([1, P], BF16)
            nc.vector.memset(ones_row, 1.0)
        if nz["bqv"]:
            bqv_sb = pp.tile([1, L, H], BF16)
            nc.sync.dma_start(out=bqv_sb, in_=bqv_d[:, :][None, :, :])
        if nz["bo"]:
            bo_sb = pp.tile([1, L, H], BF16)
            nc.sync.dma_start(out=bo_sb, in_=bo_d[:, :][None, :, :])
        if nz["b2"]:
            b2_sb = pp.tile([1, L, H], BF16)
            nc.sync.dma_start(out=b2_sb, in_=b2_d[:, :][None, :, :])
        if nz["mb2"]:
            mb2_sb = pp.tile([1, R], F32)
            nc.sync.dma_start(out=mb2_sb, in_=mb2_d[:, :])
            ones_row_f = pp.tile([1, EPC], F32)
            nc.vector.memset(ones_row_f, 1.0)
        lng_sb = None
        if nz["ln"]:
            lng_sb = pp.tile([P, 4 * L + 2, H], F32)
            nc.sync.dma_start(
                out=lng_sb,
                in_=bass.AP(tensor=lng_d, offset=0,
                            ap=[[0, P]] + lng_d[:, :].ap))

        # ------------- embedding gather -------------
        for t in range(TOT if "gather" in parts else 0):
            nc.gpsimd.indirect_dma_start(
                out=x_all[:, t, :], out_offset=None,
                in_=emb_d[:, :],
                in_offset=IndirectOffsetOnAxis(ap=ids_sb[:, t:t + 1], axis=0))

        def transpose_e(e):
            # xbar-DMA transpose of example e's token tiles, one per H-chunk
            if "trans" not in parts:
                return
            nq = QT[e]
            for c in range(2):
                nc.sync.dma_start_transpose(
                    out=xT[:, c, off[e]:off[e] + nq, :],
                    in_=x_bf[:, c, off[e]:off[e] + nq, :])

        def layernorm_e(e, ln_in, site):
            """LN over H for example e's QT[e] tiles held in ln_in [P,nq,H];
            writes result into x_all (and bf16 copy into x_bf for site<2).
            site: 0=ln1, 1=ln2 (for g/b), 2=final."""
            nq = QT[e]
            stats = sb.tile([P, 4, 6], F32, tag="lnstats")
            mv = sb.tile([P, 4, 2], F32, tag="lnmv")
            rstd = sb.tile([P, 4], F32, tag="lnrstd")
            for qt in range(nq):
                nc.vector.bn_stats(out=stats[:, qt, :], in_=ln_in[:, qt, :])
                nc.vector.bn_aggr(out=mv[:, qt, :], in_=stats[:, qt, :])
            # rstd = 1/sqrt(var+eps)
            std = sb.tile([P, 4], F32, tag="lnstd")
            nc.scalar.activation(out=std[:, :nq], in_=mv[:, :nq, 1],
                                 func=AF.Sqrt, bias=eps_col, scale=1.0)
            nc.vector.reciprocal(out=rstd[:, :nq], in_=std[:, :nq])
            for qt in range(nq):
                T = off[e] + qt
                nc.vector.tensor_scalar(
                    out=x_all[:, T, :], in0=ln_in[:, qt, :],
                    scalar1=mv[:, qt, 0:1], scalar2=rstd[:, qt:qt + 1],
                    op0=OP.subtract, op1=OP.mult)
                if site < 2 and not nz["ln"]:
                    nc.vector.tensor_scalar(
                        out=x_bf[:, :, T, :], in0=ln_in[:, qt, :].rearrange(
                            "p (c h) -> p c h", c=2),
                        scalar1=mv[:, qt, 0:1], scalar2=rstd[:, qt:qt + 1],
                        op0=OP.subtract, op1=OP.mult)
                if nz["ln"]:
                    if site == 2:
                        g_row, b_row = 4 * L, 4 * L + 1
                    else:
                        g_row = 4 * cur_l[0] + 2 * site
                        b_row = 4 * cur_l[0] + 2 * site + 1
                    nc.vector.tensor_tensor(out=x_all[:, T, :],
                                            in0=x_all[:, T, :],
                                            in1=lng_sb[:, g_row, :],
                                            op=OP.mult)
                    nc.vector.tensor_tensor(out=x_all[:, T, :],
                                            in0=x_all[:, T, :],
                                            in1=lng_sb[:, b_row, :],
                                            op=OP.add)
                    if site < 2:
                        nc.vector.tensor_copy(
                            out=x_bf[:, :, T, :],
                            in_=x_all[:, T, :].rearrange(
                                "p (c h) -> p c h", c=2))
            if site < 2:
                transpose_e(e)

        # initial bf16 cast of the embedding output
        for t in range(TOT):
            nc.vector.tensor_copy(
                out=x_bf[:, :, t, :],
                in_=x_all[:, t, :].rearrange("p (c h) -> p c h", c=2))
        for e in range(EPC):
            transpose_e(e)

        # ================= layers =================
        cur_l = [0]
        for l in range(L):
            cur_l[0] = l
            wq = wp.tile([P, 2, 3 * H], BF16, tag="wqkv")
            nc.sync.dma_start(out=wq, in_=wqkv_d[:, l, :, :])
            wo = wp.tile([P, 2, H], BF16, tag="wo")
            nc.sync.dma_start(out=wo, in_=wo_d[:, l, :, :])
            w1 = wp.tile([P, 2, FF], BF16, tag="w1")
            nc.sync.dma_start(out=w1, in_=w1_d[:, l, :, :])
            w2 = wp.tile([P, 8, H], BF16, tag="w2")
            nc.sync.dma_start(out=w2, in_=w2_d[:, l, :, :])

            # --- phase B1: attention per example (exp table resident),
            # software-pipelined: qkv(e+1) is emitted before attention(e) so
            # the PE never stalls on the q/k/v PSUM->SBUF copies ---
            def emit_qkv(e):
                kvt, nq = KVT[e], QT[e]
                ex_tok = xT[:, :, off[e]:off[e] + nq, :]  # [P,2,nq,128]
                qT = sb.tile([P, 2, S], BF16, tag="qT")
                kT = sb.tile([P, 2, S], BF16, tag="kT")
                vbf = sb.tile([P, 4, NH, DH], BF16, tag="v")
                # q, k projections (H-major)
                for (dst, col0, n) in ((qT, 0, nq), (kT, H, kvt)):
                    big = ps.tile([P, 2, S], F32, tag="big2", bufs=2)
                    for hc in range(2):
                        for c in range(2):
                            nc.tensor.matmul(
                                out=big[:, hc, :n * P],
                                lhsT=wq[:, c, col0 + hc * P:col0 + (hc + 1) * P],
                                rhs=ex_tok[:, c, :n, :],
                                start=(c == 0), stop=(c == 1))
                    for hc in range(2):
                        if nz["bqk"]:
                            col = (hc if dst is qT else 2 + hc)
                            nc.scalar.activation(
                                out=dst[:, hc, :n * P], in_=big[:, hc, :n * P],
                                func=AF.Identity,
                                bias=bqk_sb[:, l, col:col + 1], scale=1.0)
                        else:
                            nc.scalar.copy(out=dst[:, hc, :n * P],
                                           in_=big[:, hc, :n * P])
                # v projection (token-major)
                for t0 in range(0, kvt, 2):
                    bigv = ps.tile([P, 2, S], F32, tag="big2", bufs=2)
                    for t in range(t0, min(t0 + 2, kvt)):
                        for c in range(2):
                            nc.tensor.matmul(
                                out=bigv[:, t - t0, :H],
                                lhsT=ex_tok[:, c, t, :],
                                rhs=wq[:, c, 2 * H:3 * H],
                                start=(c == 0), stop=(c == 1))
                        if nz["bqv"]:
                            nc.tensor.matmul(
                                out=bigv[:, t - t0, :H], lhsT=ones_row,
                                rhs=bqv_sb[:, l, :],
                                start=False, stop=True, skip_group_check=True)
                        nc.vector.tensor_copy(out=vbf[:, t, :, :],
                                              in_=bigv[:, t - t0, :H])
                return qT, kT, vbf

            def emit_attn(e, qT, kT, vbf):
                kvt, nq = KVT[e], QT[e]
                NQ = nq * P
                for hc in range(2):
                    oav = ps.tile([P, S], F32, tag="oav", bufs=2)
                    osum = ps.tile([P, S], F32, tag="osum", bufs=2)
                    for t in range(kvt):
                        scs, exs = [], []
                        for hp in range(2):
                            sc = ps.tile([P, 2, S], F32, tag="big2", bufs=2)
                            for h2 in range(2):
                                hq = hp * 2 + h2
                                nc.tensor.matmul(
                                    out=sc[:, h2, :NQ],
                                    lhsT=kT[hq * 32:(hq + 1) * 32, hc,
                                            t * P:(t + 1) * P],
                                    rhs=qT[hq * 32:(hq + 1) * 32, hc, :NQ],
                                    start=True, stop=True,
                                    tile_position=(hq * 32, 0))
                            scs.append(sc)
                        for hp in range(2):
                            ex_t = sb.tile([P, 2, S], BF16, tag="exp", bufs=4)
                            nc.scalar.activation(out=ex_t[:, :, :NQ],
                                                 in_=scs[hp][:, :, :NQ],
                                                 func=AF.Exp,
                                                 bias=kbias_sb[:, e, t:t + 1],
                                                 scale=1.0)
                            exs.append(ex_t)
                        for hp in range(2):
                            for h2 in range(2):
                                hq = hp * 2 + h2
                                nc.tensor.matmul(
                                    out=oav[hq * 32:(hq + 1) * 32, :NQ],
                                    lhsT=vbf[:, t, hc * 4 + hq, :],
                                    rhs=exs[hp][:, h2, :NQ],
                                    start=(t == 0), stop=(t == kvt - 1),
                                    tile_position=(0, hq * 32))
                                nc.tensor.matmul(
                                    out=osum[hq * 32:(hq + 1) * 32, :NQ],
                                    lhsT=ones32, rhs=exs[hp][:, h2, :NQ],
                                    start=(t == 0), stop=(t == kvt - 1),
                                    tile_position=(0, hq * 32))
                    rec = sb.tile([P, S], F32, tag="rec")
                    nc.vector.reciprocal(out=rec[:, :NQ], in_=osum[:, :NQ])
                    nc.vector.tensor_tensor(out=oT_all[:, hc, e, :NQ],
                                            in0=oav[:, :NQ],
                                            in1=rec[:, :NQ], op=OP.mult)

            pend = None
            for e in range(EPC if "attn" in parts else 0):
                handles = emit_qkv(e)
                if pend is not None:
                    emit_attn(pend[0], *pend[1])
                pend = (e, handles)
            if pend is not None:
                emit_attn(pend[0], *pend[1])

            # scheduler fence: keep all B1 exps before B2's sqrts in the ACT
            # stream (avoids activation-table reload thrash)
            tc.no_sync_barrier()

            # --- phase B2: attn out projection + residual + LN1 ---
            for e in range(EPC if "wo" in parts else 0):
                ln_in = sb.tile([P, 4, H], F32, tag="ln_in")
                for qt in range(QT[e]):
                    wops = ps.tile([P, H], F32, tag="oav", bufs=2)
                    for hc in range(2):
                        nc.tensor.matmul(
                            out=wops,
                            lhsT=oT_all[:, hc, e, qt * P:(qt + 1) * P],
                            rhs=wo[:, hc, :],
                            start=(hc == 0), stop=(hc == 1))
                    if nz["bo"]:
                        nc.tensor.matmul(out=wops, lhsT=ones_row,
                                         rhs=bo_sb[:, l, :],
                                         start=False, stop=True,
                                         skip_group_check=True)
                    nc.vector.tensor_tensor(out=ln_in[:, qt, :], in0=wops,
                                            in1=x_all[:, off[e] + qt, :],
                                            op=OP.add)
                layernorm_e(e, ln_in, 0)

            # --- phase D: FFN per example ---
            for e in range(EPC if "ffn" in parts else 0):
                nq = QT[e]
                NQ = nq * P
                ex_tok = xT[:, :, off[e]:off[e] + nq, :]
                hT = sb.tile([P, 8, S], BF16, tag="hT")
                for g in range(4):
                    bigf = ps.tile([P, 2, S], F32, tag="big2", bufs=2)
                    for f2 in range(2):
                        fc = g * 2 + f2
                        for c in range(2):
                            nc.tensor.matmul(
                                out=bigf[:, f2, :NQ],
                                lhsT=w1[:, c, fc * P:(fc + 1) * P],
                                rhs=ex_tok[:, c, :, :],
                                start=(c == 0), stop=(c == 1))
                    for f2 in range(2):
                        fc = g * 2 + f2
                        nc.scalar.activation(
                            out=hT[:, fc, :NQ], in_=bigf[:, f2, :NQ],
                            func=AF.Relu,
                            bias=(b1_sb[:, l, fc:fc + 1] if nz["b1"]
                                  else 0.0), scale=1.0)
                ln_in = sb.tile([P, 4, H], F32, tag="ln_in")
                for qt in range(nq):
                    wops = ps.tile([P, H], F32, tag="oav", bufs=2)
                    for fc in range(8):
                        nc.tensor.matmul(
                            out=wops,
                            lhsT=hT[:, fc, qt * P:(qt + 1) * P],
                            rhs=w2[:, fc, :],
                            start=(fc == 0), stop=(fc == 7))
                    if nz["b2"]:
                        nc.tensor.matmul(out=wops, lhsT=ones_row,
                                         rhs=b2_sb[:, l, :],
                                         start=False, stop=True,
                                         skip_group_check=True)
                    nc.vector.tensor_tensor(out=ln_in[:, qt, :], in0=wops,
                                            in1=x_all[:, off[e] + qt, :],
                                            op=OP.add)
                layernorm_e(e, ln_in, 1)

        # ================ final LN + pools + head ================
        for e in range(EPC if "head" in parts else 0):
            ln_in = sb.tile([P, 4, H], F32, tag="ln_in")
            for qt in range(QT[e]):
                nc.vector.tensor_copy(out=ln_in[:, qt, :],
                                      in_=x_all[:, off[e] + qt, :])
            layernorm_e(e, ln_in, 2)

        for e in range(EPC if "head" in parts else 0):
            for hc in range(2):
                plp = ps.tile([P, 3], F32, tag="oav", bufs=2)
                for t in range(QT[e]):
                    nc.tensor.matmul(
                        out=plp,
                        lhsT=x_all[:, off[e] + t, hc * P:(hc + 1) * P],
                        rhs=wpool_sb[:, e, t, :],
                        start=(t == 0), stop=(t == QT[e] - 1))
                nc.vector.tensor_copy(out=featT[:, :, hc, e], in_=plp)

        if "head" not in parts:
            nc.vector.memset(featT, 0.0)
        h1T = pp.tile([P, 2, EPC], F32)
        for m in range(2):
            hps = ps.tile([P, EPC], F32, tag="oav", bufs=2)
            for c in range(6):
                j, hc = c // 2, c % 2
                nc.tensor.matmul(out=hps,
                                 lhsT=mw1_sb[:, c, m * P:(m + 1) * P],
                                 rhs=featT[:, j, hc, :],
                                 start=(c == 0), stop=(c == 5))
            nc.scalar.activation(
                out=h1T[:, m, :], in_=hps, func=AF.Relu,
                bias=(mb1_sb[:, m:m + 1] if nz["mb1"] else 0.0), scale=1.0)

        lps = ps.tile([EPC, R], F32, tag="oav", bufs=2)
        for m in range(2):
            nc.tensor.matmul(out=lps, lhsT=h1T[:, m, :], rhs=mw2_sb[:, m, :],
                             start=(m == 0), stop=(m == 1))
        if nz["mb2"]:
            nc.tensor.matmul(out=lps, lhsT=ones_row_f, rhs=mb2_sb,
                             start=False, stop=True, skip_group_check=True)
        out_sb = sb.tile([EPC, R], F32, tag="out")
        nc.vector.tensor_copy(out=out_sb, in_=lps)
        nc.sync.dma_start(out=out_d[:, :], in_=out_sb)

    nc.compile()
    return nc


def _prep_weights(inputs, nz):
    """Host-side weight repacking shared by all cores."""
    scale = 1.0 / np.sqrt(DH)
    wqkv = np.array(inputs["Wqkv"], dtype=np.float32)  # [L,H,3H]
    wqkv[:, :, :H] *= scale
    d = {}
    d["emb"] = np.ascontiguousarray(inputs["emb"], dtype=np.float32)
    # [P, L, 2, 3H]: [l, c*128+p, j] -> [p, l, c, j]
    d["wqkv"] = _bf(wqkv.reshape(L, 2, P, 3 * H).transpose(2, 0, 1, 3))
    d["wo"] = _bf(np.asarray(inputs["Wo"], np.float32)
                  .reshape(L, 2, P, H).transpose(2, 0, 1, 3))
    d["w1"] = _bf(np.asarray(inputs["W1"], np.float32)
                  .reshape(L, 2, P, FF).transpose(2, 0, 1, 3))
    d["w2"] = _bf(np.asarray(inputs["W2"], np.float32)
                  .reshape(L, 8, P, H).transpose(2, 0, 1, 3))
    d["mw1"] = np.ascontiguousarray(
        np.asarray(inputs["mW1"], np.float32).reshape(6, P, H)
        .transpose(1, 0, 2))
    d["mw2"] = np.ascontiguousarray(
        np.asarray(inputs["mW2"], np.float32).reshape(2, P, R)
        .transpose(1, 0, 2))
    if nz["bqk"]:
        bq = np.asarray(inputs["bqkv"], np.float32)[:, :H] * scale
        bk = np.asarray(inputs["bqkv"], np.float32)[:, H:2 * H]
        arr = np.zeros((P, L, 4), np.float32)
        for l in range(L):
            arr[:, l, 0] = bq[l, :P]
            arr[:, l, 1] = bq[l, P:]
            arr[:, l, 2] = bk[l, :P]
            arr[:, l, 3] = bk[l, P:]
        d["bqk"] = arr
    if nz["bqv"]:
        d["bqv"] = _bf(np.asarray(inputs["bqkv"], np.float32)[:, 2 * H:])
    if nz["b1"]:
        d["b1"] = np.ascontiguousarray(
            np.asarray(inputs["b1"], np.float32).reshape(L, 8, P)
            .transpose(2, 0, 1))
    if nz["bo"]:
        d["bo"] = _bf(inputs["bo"])
    if nz["b2"]:
        d["b2"] = _bf(inputs["b2"])
    if nz["mb1"]:
        d["mb1"] = np.ascontiguousarray(
            np.asarray(inputs["mb1"], np.float32).reshape(2, P).T)
    if nz["mb2"]:
        d["mb2"] = np.asarray(inputs["mb2"], np.float32).reshape(1, R)
    if nz["ln"]:
        rows = []
        for l in range(L):
            rows += [inputs["ln1_g"][l], inputs["ln1_b"][l],
                     inputs["ln2_g"][l], inputs["ln2_b"][l]]
        rows += [inputs["lnf_g"], inputs["lnf_b"]]
        d["lngb"] = np.asarray(np.stack(rows), np.float32)
    return d


def prepare(inputs):
    """Host-side sharding/prep. Returns (in_maps, assign, KVT, nz)."""
    input_ids = np.asarray(inputs["input_ids"])
    attention_mask = np.asarray(inputs["attention_mask"])
    assert input_ids.shape == (B, S)

    nz = {
        "bqk": np.any(np.asarray(inputs["bqkv"], np.float32)[:, :2 * H] != 0),
        "bqv": np.any(np.asarray(inputs["bqkv"], np.float32)[:, 2 * H:] != 0),
        "bo": np.any(np.asarray(inputs["bo"]) != 0),
        "b1": np.any(np.asarray(inputs["b1"]) != 0),
        "b2": np.any(np.asarray(inputs["b2"]) != 0),
        "mb1": np.any(np.asarray(inputs["mb1"]) != 0),
        "mb2": np.any(np.asarray(inputs["mb2"]) != 0),
        "ln": not all(
            np.all(np.asarray(inputs[k]) == 1) for k in
            ("ln1_g", "ln2_g", "lnf_g")) or not all(
            np.all(np.asarray(inputs[k]) == 0) for k in
            ("ln1_b", "ln2_b", "lnf_b")),
    }

    lengths = attention_mask.astype(np.int64).sum(1)  # [B]
    # active token set per example: valid prefix + any sub/obj gather target
    actives = []
    for ex in range(B):
        act = set(range(int(lengths[ex])))
        c = int(inputs["sub_counts"][ex])
        act |= set(np.asarray(inputs["sub_indices"][ex][:c], np.int64).tolist())
        c = int(inputs["obj_counts"][ex])
        act |= set(np.asarray(inputs["obj_indices"][ex][:c], np.int64).tolist())
        if not act:
            act = {0}
        actives.append(np.array(sorted(act), dtype=np.int64))
    n_act = np.array([len(a) for a in actives])
    # snake-deal sorted examples: slot j on core c gets rank j*8 + (j even? c : 7-c)
    order = np.argsort(-lengths, kind="stable")
    assign = np.empty((NCORES, EPC), dtype=np.int64)
    for j in range(EPC):
        grp = order[j * NCORES:(j + 1) * NCORES]
        assign[:, j] = grp if j % 2 == 0 else grp[::-1]
    KVT = [max(1, int(np.ceil(lengths[assign[:, j]].max() / 128.0)))
           for j in range(EPC)]
    QT = [max(KVT[j], int(np.ceil(n_act[assign[:, j]].max() / 128.0)))
          for j in range(EPC)]
    off = [sum(QT[:e]) for e in range(EPC)]
    TOT = sum(QT)

    shared = _prep_weights(inputs, nz)

    in_maps = []
    for c in range(NCORES):
        exs = assign[c]
        ids_c = np.zeros((TOT * P,), np.int32)
        kb_c = np.full((EPC, 4 * P), -1e9, np.float32)
        wpool_c = np.zeros((EPC, 4 * P, 3), np.float32)
        for j, ex in enumerate(exs):
            A = actives[ex]
            npad = QT[j] * P
            toks = np.zeros((npad,), np.int64)
            toks[:len(A)] = A                      # pad with token position 0
            ids_c[off[j] * P:off[j] * P + npad] = input_ids[ex][toks]
            ln = int(lengths[ex])
            kb_c[j, :ln] = 0.0
            # pool weights in compacted coordinates
            pos = {int(o): i for i, o in enumerate(A)}
            cnt = int(inputs["sub_counts"][ex])
            if cnt > 0:
                idxs = [pos[int(i)] for i in
                        np.asarray(inputs["sub_indices"][ex][:cnt], np.int64)]
                np.add.at(wpool_c[j, :, 0], idxs, 1.0 / cnt)
            cnt = int(inputs["obj_counts"][ex])
            if cnt > 0:
                idxs = [pos[int(i)] for i in
                        np.asarray(inputs["obj_indices"][ex][:cnt], np.int64)]
                np.add.at(wpool_c[j, :, 1], idxs, 1.0 / cnt)
            if ln > 0:
                wpool_c[j, :ln, 2] = 1.0 / ln
            else:
                # reference falls back to plain mean over all S positions;
                # with ln==0 every position is padding - approximate with
                # uniform weights over the compacted set (values identical
                # rows, so result matches mean over duplicates of row 0...
                # cannot happen since lengths >= 1 in practice)
                wpool_c[j, :, 2] = 1.0 / npad
        m = dict(shared)
        m["ids"] = np.ascontiguousarray(ids_c.reshape(TOT, P).T)  # [P, TOT]
        m["kbias"] = np.ascontiguousarray(
            kb_c.reshape(EPC, 4, P).transpose(2, 0, 1))   # [P, EPC, 4]
        m["wpool"] = np.ascontiguousarray(
            wpool_c.reshape(EPC, 4, P, 3).transpose(2, 0, 1, 3))
        in_maps.append(m)
    return in_maps, assign, KVT, QT, nz


def kernel(**inputs):
    in_maps, assign, KVT, QT, nz = prepare(inputs)
    nc = build_program_cached(KVT, QT, nz)
    res = run_bass_kernel_spmd(nc, in_maps, core_ids=list(range(NCORES)))
    logits = np.zeros((B, R), np.float32)
    for c in range(NCORES):
        logits[assign[c]] = res.results[c]["out"]
    return logits


if __name__ == "__main__":
    np.random.seed(0)
    fake = {}
    rng = np.random.default_rng(0)
    fake["input_ids"] = rng.integers(0, V, (B, S)).astype(np.int64)
    ln_ = rng.integers(16, S + 1, (B,))
    fake["attention_mask"] = (np.arange(S)[None] < ln_[:, None]).astype(np.int32)
    fake["sub_indices"] = rng.integers(0, S, (B, KIDX)).astype(np.int64)
    fake["obj_indices"] = rng.integers(0, S, (B, KIDX)).astype(np.int64)
    fake["sub_counts"] = rng.integers(0, KIDX + 1, (B,)).astype(np.int32)
    fake["obj_counts"] = rng.integers(0, KIDX + 1, (B,)).astype(np.int32)
    sc = 0.02
    fake["emb"] = rng.normal(0, sc, (V, H)).astype(np.float32)
    fake["Wqkv"] = rng.normal(0, sc, (L, H, 3 * H)).astype(np.float32)
    fake["bqkv"] = np.zeros((L, 3 * H), np.float32)
    fake["Wo"] = rng.normal(0, sc, (L, H, H)).astype(np.float32)
    fake["bo"] = np.zeros((L, H), np.float32)
    fake["ln1_g"] = np.ones((L, H), np.float32)
    fake["ln1_b"] = np.zeros((L, H), np.float32)
    fake["ln2_g"] = np.ones((L, H), np.float32)
    fake["ln2_b"] = np.zeros((L, H), np.float32)
    fake["W1"] = rng.normal(0, sc, (L, H, FF)).astype(np.float32)
    fake["b1"] = np.zeros((L, FF), np.float32)
    fake["W2"] = rng.normal(0, sc, (L, FF, H)).astype(np.float32)
    fake["b2"] = np.zeros((L, H), np.float32)
    fake["lnf_g"] = np.ones((H,), np.float32)
    fake["lnf_b"] = np.zeros((H,), np.float32)
    fake["mW1"] = rng.normal(0, sc, (3 * H, H)).astype(np.float32)
    fake["mb1"] = np.zeros((H,), np.float32)
    fake["mW2"] = rng.normal(0, sc, (H, R)).astype(np.float32)
    fake["mb2"] = np.zeros((R,), np.float32)
    out = kernel(**fake)
    print(out.shape, out.dtype, np.abs(out).max())



# revision 22
# speedup vs baseline: 10.3656x; 10.3656x over previous
"""Trainium2 Bass kernel for a 4-layer CLUTRR transformer encoder + pooling head.

Strategy: data-parallel over batch (8 examples per core x 8 cores).
- Examples are sorted by valid length and dealt snake-wise so every core's
  slot j has a similar kv length; the program is specialized per slot to
  skip attention work beyond ceil(maxlen/128) key tiles.
- Residual stream kept in SBUF fp32 token-major; per-layer bf16 copies +
  DMA-xbar transposes provide the H-major operands for matmuls.
- Attention computed in "scoresT" layout [k_part, q_free]: key-side mask is a
  per-partition bias on the Exp activation; softmax denominator comes from a
  col-tiled ones-matmul; normalization folded into the PSUM->SBUF copy.
- All big matmuls run in bf16 (fp32 PSUM accumulation).
"""

import sys

sys.path.insert(0, "/opt/trn_rl_repo")

import numpy as np
import ml_dtypes

import concourse.bass as bass
import concourse.tile as tile
from concourse import bacc, mybir
from concourse.bass import IndirectOffsetOnAxis
from concourse.bass_utils import run_bass_kernel_spmd

F32 = mybir.dt.float32
BF16 = mybir.dt.bfloat16
I32 = mybir.dt.int32
AF = mybir.ActivationFunctionType
OP = mybir.AluOpType

B, S, H, L, NH, FF, V, KIDX, R = 64, 512, 256, 4, 8, 1024, 50265, 8, 25
DH = H // NH
EPS = 1e-5
NCORES = 8
EPC = B // NCORES  # examples per core
TPC = EPC * S // 128  # token tiles per core (32)
P = 128


def _bf(x):
    return np.asarray(x, dtype=np.float32).astype(ml_dtypes.bfloat16)


_PROG_CACHE = {}


def build_program_cached(KVT, QT, nz, reps=1, parts=None):
    key = (tuple(KVT), tuple(QT), tuple(sorted(nz.items())), reps,
           tuple(sorted(parts)) if parts else None)
    if key not in _PROG_CACHE:
        _PROG_CACHE[key] = build_program(KVT, QT, nz, reps=reps, parts=parts)
    return _PROG_CACHE[key]


def build_program(KVT, QT, nz, reps=1, parts=None):
    if parts is None:
        parts = {"gather", "trans", "attn", "wo", "ffn", "head"}
    """Build the per-core Bass program.

    KVT: list of EPC ints - number of valid 128-key tiles per example slot.
    QT: list of EPC ints - number of active (compacted) token tiles per slot.
    nz: dict of flags for which optional params are nonzero/non-identity.
    """
    TOT = sum(QT)                      # total compacted token tiles per core
    off = [sum(QT[:e]) for e in range(EPC)]
    assert all(KVT[e] <= QT[e] for e in range(EPC))

    nc = bacc.Bacc("TRN2", target_bir_lowering=False, debug=False,
                   num_devices=NCORES)

    # ---------------- DRAM tensors ----------------
    emb_d = nc.dram_tensor("emb", [V, H], F32, kind="ExternalInput")
    ids_d = nc.dram_tensor("ids", [P, TOT], I32, kind="ExternalInput")
    kbias_d = nc.dram_tensor("kbias", [P, EPC, 4], F32, kind="ExternalInput")
    wpool_d = nc.dram_tensor("wpool", [P, EPC, 4, 3], F32, kind="ExternalInput")
    wqkv_d = nc.dram_tensor("wqkv", [P, L, 2, 3 * H], BF16, kind="ExternalInput")
    wo_d = nc.dram_tensor("wo", [P, L, 2, H], BF16, kind="ExternalInput")
    w1_d = nc.dram_tensor("w1", [P, L, 2, FF], BF16, kind="ExternalInput")
    w2_d = nc.dram_tensor("w2", [P, L, 8, H], BF16, kind="ExternalInput")
    mw1_d = nc.dram_tensor("mw1", [P, 6, H], F32, kind="ExternalInput")
    mw2_d = nc.dram_tensor("mw2", [P, 2, R], F32, kind="ExternalInput")
    out_d = nc.dram_tensor("out", [EPC, R], F32, kind="ExternalOutput")

    # optional (nonzero) parameter tensors
    bqk_d = bqv_d = b1_d = bo_d = b2_d = mb1_d = mb2_d = lng_d = None
    if nz["bqk"]:
        bqk_d = nc.dram_tensor("bqk", [P, L, 4], F32, kind="ExternalInput")
    if nz["bqv"]:
        bqv_d = nc.dram_tensor("bqv", [L, H], BF16, kind="ExternalInput")
    if nz["b1"]:
        b1_d = nc.dram_tensor("b1", [P, L, 8], F32, kind="ExternalInput")
    if nz["bo"]:
        bo_d = nc.dram_tensor("bo", [L, H], BF16, kind="ExternalInput")
    if nz["b2"]:
        b2_d = nc.dram_tensor("b2", [L, H], BF16, kind="ExternalInput")
    if nz["mb1"]:
        mb1_d = nc.dram_tensor("mb1", [P, 2], F32, kind="ExternalInput")
    if nz["mb2"]:
        mb2_d = nc.dram_tensor("mb2", [1, R], F32, kind="ExternalInput")
    if nz["ln"]:
        # 9 rows: ln1_g,ln1_b,ln2_g,ln2_b per layer packed [4L+2, H] on host
        lng_d = nc.dram_tensor("lngb", [4 * L + 2, H], F32, kind="ExternalInput")

    from contextlib import ExitStack, nullcontext
    with tile.TileContext(nc) as tc, ExitStack() as ctx:
        pp = ctx.enter_context(tc.tile_pool(name="persist", bufs=1))
        wp = ctx.enter_context(tc.tile_pool(name="weights", bufs=2))
        sb = ctx.enter_context(tc.tile_pool(name="work", bufs=3))
        ps = ctx.enter_context(tc.tile_pool(name="psum", bufs=1, space="PSUM"))
        rep_cm = tc.For_i(0, reps, 1) if reps > 1 else nullcontext()
        ctx.enter_context(rep_cm)

        # ------------- persistent tiles -------------
        x_all = pp.tile([P, TOT, H], F32)     # residual stream, token-major
        x_bf = pp.tile([P, 2, TOT, P], BF16)  # bf16 cast, [tok, chunk, tile, h]
        xT = pp.tile([P, 2, TOT, P], BF16)    # H-major bf16 [h, chunk, tile, tok]
        oT_all = pp.tile([P, 2, EPC, S], BF16)  # attn out, H-major per example
        ids_sb = pp.tile([P, TOT], I32)
        kbias_sb = pp.tile([P, EPC, 4], F32)
        wpool_sb = pp.tile([P, EPC, 4, 3], F32)
        ones32 = pp.tile([P, 32], BF16)
        eps_col = pp.tile([P, 1], F32)
        mw1_sb = pp.tile([P, 6, H], F32)
        mw2_sb = pp.tile([P, 2, R], F32)
        featT = pp.tile([P, 3, 2, EPC], F32)  # [h, j(sub/obj/story), hc, e]

        nc.sync.dma_start(out=ids_sb, in_=ids_d[:, :])
        nc.sync.dma_start(out=kbias_sb, in_=kbias_d[:, :, :])
        nc.sync.dma_start(out=wpool_sb, in_=wpool_d[:, :, :, :])
        nc.sync.dma_start(out=mw1_sb, in_=mw1_d[:, :, :])
        nc.sync.dma_start(out=mw2_sb, in_=mw2_d[:, :, :])
        nc.vector.memset(ones32, 1.0)
        nc.vector.memset(eps_col, EPS)

        bqk_sb = bqv_sb = b1_sb = bo_sb = b2_sb = mb1_sb = mb2_sb = None
        ones_row = None  # [1,128] bf16 for K=1 bias matmuls
        if nz["bqk"]:
            bqk_sb = pp.tile([P, L, 4], F32)
            nc.sync.dma_start(out=bqk_sb, in_=bqk_d[:, :, :])
        if nz["b1"]:
            b1_sb = pp.tile([P, L, 8], F32)
            nc.sync.dma_start(out=b1_sb, in_=b1_d[:, :, :])
        if nz["mb1"]:
            mb1_sb = pp.tile([P, 2], F32)
            nc.sync.dma_start(out=mb1_sb, in_=mb1_d[:, :])
        if nz["bqv"] or nz["bo"] or nz["b2"]:
            ones_row = pp.tile([1, P], BF16)
            nc.vector.memset(ones_row, 1.0)
        if nz["bqv"]:
            bqv_sb = pp.tile([1, L, H], BF16)
            nc.sync.dma_start(out=bqv_sb, in_=bqv_d[:, :][None, :, :])
        if nz["bo"]:
            bo_sb = pp.tile([1, L, H], BF16)
            nc.sync.dma_start(out=bo_sb, in_=bo_d[:, :][None, :, :])
        if nz["b2"]:
            b2_sb = pp.tile([1, L, H], BF16)
            nc.sync.dma_start(out=b2_sb, in_=b2_d[:, :][None, :, :])
        if nz["mb2"]:
            mb2_sb = pp.tile([1, R], F32)
            nc.sync.dma_start(out=mb2_sb, in_=mb2_d[:, :])
            ones_row_f = pp.tile([1, EPC], F32)
            nc.vector.memset(ones_row_f, 1.0)
        lng_sb = None
        if nz["ln"]:
            lng_sb = pp.tile([P, 4 * L + 2, H], F32)
            nc.sync.dma_start(
                out=lng_sb,
                in_=bass.AP(tensor=lng_d, offset=0,
                            ap=[[0, P]] + lng_d[:, :].ap))

        # ------------- embedding gather -------------
        for t in range(TOT if "gather" in parts else 0):
            nc.gpsimd.indirect_dma_start(
                out=x_all[:, t, :], out_offset=None,
                in_=emb_d[:, :],
                in_offset=IndirectOffsetOnAxis(ap=ids_sb[:, t:t + 1], axis=0))

        def transpose_e(e):
            # xbar-DMA transpose of example e's token tiles, one per H-chunk
            if "trans" not in parts:
                return
            nq = QT[e]
            for c in range(2):
                nc.sync.dma_start_transpose(
                    out=xT[:, c, off[e]:off[e] + nq, :],
                    in_=x_bf[:, c, off[e]:off[e] + nq, :])

        def layernorm_e(e, ln_in, site):
            """LN over H for example e's QT[e] tiles held in ln_in [P,nq,H];
            writes result into x_all (and bf16 copy into x_bf for site<2).
            site: 0=ln1, 1=ln2 (for g/b), 2=final."""
            nq = QT[e]
            stats = sb.tile([P, 4, 6], F32, tag="lnstats")
            mv = sb.tile([P, 4, 2], F32, tag="lnmv")
            rstd = sb.tile([P, 4], F32, tag="lnrstd")
            for qt in range(nq):
                nc.vector.bn_stats(out=stats[:, qt, :], in_=ln_in[:, qt, :])
                nc.vector.bn_aggr(out=mv[:, qt, :], in_=stats[:, qt, :])
            # rstd = 1/sqrt(var+eps)
            std = sb.tile([P, 4], F32, tag="lnstd")
            nc.scalar.activation(out=std[:, :nq], in_=mv[:, :nq, 1],
                                 func=AF.Sqrt, bias=eps_col, scale=1.0)
            nc.vector.reciprocal(out=rstd[:, :nq], in_=std[:, :nq])
            for qt in range(nq):
                T = off[e] + qt
                nc.vector.tensor_scalar(
                    out=x_all[:, T, :], in0=ln_in[:, qt, :],
                    scalar1=mv[:, qt, 0:1], scalar2=rstd[:, qt:qt + 1],
                    op0=OP.subtract, op1=OP.mult)
                if site < 2 and not nz["ln"]:
                    nc.vector.tensor_scalar(
                        out=x_bf[:, :, T, :], in0=ln_in[:, qt, :].rearrange(
                            "p (c h) -> p c h", c=2),
                        scalar1=mv[:, qt, 0:1], scalar2=rstd[:, qt:qt + 1],
                        op0=OP.subtract, op1=OP.mult)
                if nz["ln"]:
                    if site == 2:
                        g_row, b_row = 4 * L, 4 * L + 1
                    else:
                        g_row = 4 * cur_l[0] + 2 * site
                        b_row = 4 * cur_l[0] + 2 * site + 1
                    nc.vector.tensor_tensor(out=x_all[:, T, :],
                                            in0=x_all[:, T, :],
                                            in1=lng_sb[:, g_row, :],
                                            op=OP.mult)
                    nc.vector.tensor_tensor(out=x_all[:, T, :],
                                            in0=x_all[:, T, :],
                                            in1=lng_sb[:, b_row, :],
                                            op=OP.add)
                    if site < 2:
                        nc.vector.tensor_copy(
                            out=x_bf[:, :, T, :],
                            in_=x_all[:, T, :].rearrange(
                                "p (c h) -> p c h", c=2))
            if site < 2:
                transpose_e(e)

        # initial bf16 cast of the embedding output
        for t in range(TOT):
            nc.vector.tensor_copy(
                out=x_bf[:, :, t, :],
                in_=x_all[:, t, :].rearrange("p (c h) -> p c h", c=2))
        for e in range(EPC):
            transpose_e(e)

        # ================= layers =================
        cur_l = [0]
        for l in range(L):
            cur_l[0] = l
            wq = wp.tile([P, 2, 3 * H], BF16, tag="wqkv")
            nc.sync.dma_start(out=wq, in_=wqkv_d[:, l, :, :])
            wo = wp.tile([P, 2, H], BF16, tag="wo")
            nc.sync.dma_start(out=wo, in_=wo_d[:, l, :, :])
            w1 = wp.tile([P, 2, FF], BF16, tag="w1")
            nc.sync.dma_start(out=w1, in_=w1_d[:, l, :, :])
            w2 = wp.tile([P, 8, H], BF16, tag="w2")
            nc.sync.dma_start(out=w2, in_=w2_d[:, l, :, :])

            # --- phase B1: attention per example (exp table resident),
            # software-pipelined: qkv(e+1) is emitted before attention(e) so
            # the PE never stalls on the q/k/v PSUM->SBUF copies ---
            def emit_qkv(e):
                kvt, nq = KVT[e], QT[e]
                ex_tok = xT[:, :, off[e]:off[e] + nq, :]  # [P,2,nq,128]
                qT = sb.tile([P, 2, S], BF16, tag="qT")
                kT = sb.tile([P, 2, S], BF16, tag="kT")
                vbf = sb.tile([P, 4, NH, DH], BF16, tag="v")
                # q, k projections (H-major)
                for (dst, col0, n) in ((qT, 0, nq), (kT, H, kvt)):
                    big = ps.tile([P, 2, S], F32, tag="big2", bufs=2)
                    for hc in range(2):
                        for c in range(2):
                            nc.tensor.matmul(
                                out=big[:, hc, :n * P],
                                lhsT=wq[:, c, col0 + hc * P:col0 + (hc + 1) * P],
                                rhs=ex_tok[:, c, :n, :],
                                start=(c == 0), stop=(c == 1))
                    for hc in range(2):
                        if nz["bqk"]:
                            col = (hc if dst is qT else 2 + hc)
                            nc.scalar.activation(
                                out=dst[:, hc, :n * P], in_=big[:, hc, :n * P],
                                func=AF.Identity,
                                bias=bqk_sb[:, l, col:col + 1], scale=1.0)
                        else:
                            nc.scalar.copy(out=dst[:, hc, :n * P],
                                           in_=big[:, hc, :n * P])
                # v projection (token-major)
                for t0 in range(0, kvt, 2):
                    bigv = ps.tile([P, 2, S], F32, tag="big2", bufs=2)
                    for t in range(t0, min(t0 + 2, kvt)):
                        for c in range(2):
                            nc.tensor.matmul(
                                out=bigv[:, t - t0, :H],
                                lhsT=ex_tok[:, c, t, :],
                                rhs=wq[:, c, 2 * H:3 * H],
                                start=(c == 0), stop=(c == 1))
                        if nz["bqv"]:
                            nc.tensor.matmul(
                                out=bigv[:, t - t0, :H], lhsT=ones_row,
                                rhs=bqv_sb[:, l, :],
                                start=False, stop=True, skip_group_check=True)
                        nc.vector.tensor_copy(out=vbf[:, t, :, :],
                                              in_=bigv[:, t - t0, :H])
                return qT, kT, vbf

            def emit_attn(e, qT, kT, vbf):
                kvt, nq = KVT[e], QT[e]
                NQ = nq * P
                for hc in range(2):
                    oav = ps.tile([P, S], F32, tag="oav", bufs=2)
                    osum = ps.tile([P, S], F32, tag="osum", bufs=2)
                    for t in range(kvt):
                        scs, exs = [], []
                        for hp in range(2):
                            sc = ps.tile([P, 2, S], F32, tag="big2", bufs=2)
                            for h2 in range(2):
                                hq = hp * 2 + h2
                                nc.tensor.matmul(
                                    out=sc[:, h2, :NQ],
                                    lhsT=kT[hq * 32:(hq + 1) * 32, hc,
                                            t * P:(t + 1) * P],
                                    rhs=qT[hq * 32:(hq + 1) * 32, hc, :NQ],
                                    start=True, stop=True,
                                    tile_position=(hq * 32, 0))
                            scs.append(sc)
                        for hp in range(2):
                            ex_t = sb.tile([P, 2, S], BF16, tag="exp", bufs=4)
                            nc.scalar.activation(out=ex_t[:, :, :NQ],
                                                 in_=scs[hp][:, :, :NQ],
                                                 func=AF.Exp,
                                                 bias=kbias_sb[:, e, t:t + 1],
                                                 scale=1.0)
                            exs.append(ex_t)
                        for hp in range(2):
                            for h2 in range(2):
                                hq = hp * 2 + h2
                                nc.tensor.matmul(
                                    out=oav[hq * 32:(hq + 1) * 32, :NQ],
                                    lhsT=vbf[:, t, hc * 4 + hq, :],
                                    rhs=exs[hp][:, h2, :NQ],
                                    start=(t == 0), stop=(t == kvt - 1),
                                    tile_position=(0, hq * 32))
                                nc.tensor.matmul(
                                    out=osum[hq * 32:(hq + 1) * 32, :NQ],
                                    lhsT=ones32, rhs=exs[hp][:, h2, :NQ],
                                    start=(t == 0), stop=(t == kvt - 1),
                                    tile_position=(0, hq * 32))
                    rec = sb.tile([P, S], F32, tag="rec")
                    nc.vector.reciprocal(out=rec[:, :NQ], in_=osum[:, :NQ])
                    nc.vector.tensor_tensor(out=oT_all[:, hc, e, :NQ],
                                            in0=oav[:, :NQ],
                                            in1=rec[:, :NQ], op=OP.mult)

            pend = None
            for e in range(EPC if "attn" in parts else 0):
                handles = emit_qkv(e)
                if pend is not None:
                    emit_attn(pend[0], *pend[1])
                pend = (e, handles)
            if pend is not None:
                emit_attn(pend[0], *pend[1])

            # scheduler fence: keep all B1 exps before B2's sqrts in the ACT
            # stream (avoids activation-table reload thrash)
            tc.no_sync_barrier()

            # --- phase B2: attn out projection + residual + LN1 ---
            for e in range(EPC if "wo" in parts else 0):
                ln_in = sb.tile([P, 4, H], F32, tag="ln_in")
                for qt in range(QT[e]):
                    wops = ps.tile([P, H], F32, tag="oav", bufs=2)
                    for hc in range(2):
                        nc.tensor.matmul(
                            out=wops,
                            lhsT=oT_all[:, hc, e, qt * P:(qt + 1) * P],
                            rhs=wo[:, hc, :],
                            start=(hc == 0), stop=(hc == 1))
                    if nz["bo"]:
                        nc.tensor.matmul(out=wops, lhsT=ones_row,
                                         rhs=bo_sb[:, l, :],
                                         start=False, stop=True,
                                         skip_group_check=True)
                    nc.vector.tensor_tensor(out=ln_in[:, qt, :], in0=wops,
                                            in1=x_all[:, off[e] + qt, :],
                                            op=OP.add)
                layernorm_e(e, ln_in, 0)

            # --- phase D: FFN per example ---
            for e in range(EPC if "ffn" in parts else 0):
                nq = QT[e]
                NQ = nq * P
                ex_tok = xT[:, :, off[e]:off[e] + nq, :]
                hT = sb.tile([P, 8, S], BF16, tag="hT")
                for g in range(4):
                    bigf = ps.tile([P, 2, S], F32, tag="big2", bufs=2)
                    for f2 in range(2):
                        fc = g * 2 + f2
                        for c in range(2):
                            nc.tensor.matmul(
                                out=bigf[:, f2, :NQ],
                                lhsT=w1[:, c, fc * P:(fc + 1) * P],
                                rhs=ex_tok[:, c, :, :],
                                start=(c == 0), stop=(c == 1))
                    for f2 in range(2):
                        fc = g * 2 + f2
                        nc.scalar.activation(
                            out=hT[:, fc, :NQ], in_=bigf[:, f2, :NQ],
                            func=AF.Relu,
                            bias=(b1_sb[:, l, fc:fc + 1] if nz["b1"]
                                  else 0.0), scale=1.0)
                ln_in = sb.tile([P, 4, H], F32, tag="ln_in")
                for qt in range(nq):
                    wops = ps.tile([P, H], F32, tag="oav", bufs=2)
                    for fc in range(8):
                        nc.tensor.matmul(
                            out=wops,
                            lhsT=hT[:, fc, qt * P:(qt + 1) * P],
                            rhs=w2[:, fc, :],
                            start=(fc == 0), stop=(fc == 7))
                    if nz["b2"]:
                        nc.tensor.matmul(out=wops, lhsT=ones_row,
                                         rhs=b2_sb[:, l, :],
                                         start=False, stop=True,
                                         skip_group_check=True)
                    nc.vector.tensor_tensor(out=ln_in[:, qt, :], in0=wops,
                                            in1=x_all[:, off[e] + qt, :],
                                            op=OP.add)
                layernorm_e(e, ln_in, 1)

        # ================ final LN + pools + head ================
        for e in range(EPC if "head" in parts else 0):
            ln_in = sb.tile([P, 4, H], F32, tag="ln_in")
            for qt in range(QT[e]):
                nc.vector.tensor_copy(out=ln_in[:, qt, :],
                                      in_=x_all[:, off[e] + qt, :])
            layernorm_e(e, ln_in, 2)

        for e in range(EPC if "head" in parts else 0):
            for hc in range(2):
                plp = ps.tile([P, 3], F32, tag="oav", bufs=2)
                for t in range(QT[e]):
                    nc.tensor.matmul(
                        out=plp,
                        lhsT=x_all[:, off[e] + t, hc * P:(hc + 1) * P],
                        rhs=wpool_sb[:, e, t, :],
                        start=(t == 0), stop=(t == QT[e] - 1))
                nc.vector.tensor_copy(out=featT[:, :, hc, e], in_=plp)

        if "head" not in parts:
            nc.vector.memset(featT, 0.0)
        h1T = pp.tile([P, 2, EPC], F32)
        for m in range(2):
            hps = ps.tile([P, EPC], F32, tag="oav", bufs=2)
            for c in range(6):
                j, hc = c // 2, c % 2
                nc.tensor.matmul(out=hps,
                                 lhsT=mw1_sb[:, c, m * P:(m + 1) * P],
                                 rhs=featT[:, j, hc, :],
                                 start=(c == 0), stop=(c == 5))
            nc.scalar.activation(
                out=h1T[:, m, :], in_=hps, func=AF.Relu,
                bias=(mb1_sb[:, m:m + 1] if nz["mb1"] else 0.0), scale=1.0)

        lps = ps.tile([EPC, R], F32, tag="oav", bufs=2)
        for m in range(2):
            nc.tensor.matmul(out=lps, lhsT=h1T[:, m, :], rhs=mw2_sb[:, m, :],
                             start=(m == 0), stop=(m == 1))
        if nz["mb2"]:
            nc.tensor.matmul(out=lps, lhsT=ones_row_f, rhs=mb2_sb,
                             start=False, stop=True, skip_group_check=True)
        out_sb = sb.tile([EPC, R], F32, tag="out")
        nc.vector.tensor_copy(out=out_sb, in_=lps)
        nc.sync.dma_start(out=out_d[:, :], in_=out_sb)

    nc.compile()
    return nc


def _prep_weights(inputs, nz):
    """Host-side weight repacking shared by all cores."""
    scale = 1.0 / np.sqrt(DH)
    wqkv = np.array(inputs["Wqkv"], dtype=np.float32)  # [L,H,3H]
    wqkv[:, :, :H] *= scale
    d = {}
    d["emb"] = np.ascontiguousarray(inputs["emb"], dtype=np.float32)
    # [P, L, 2, 3H]: [l, c*128+p, j] -> [p, l, c, j]
    d["wqkv"] = _bf(wqkv.reshape(L, 2, P, 3 * H).transpose(2, 0, 1, 3))
    d["wo"] = _bf(np.asarray(inputs["Wo"], np.float32)
                  .reshape(L, 2, P, H).transpose(2, 0, 1, 3))
    d["w1"] = _bf(np.asarray(inputs["W1"], np.float32)
                  .reshape(L, 2, P, FF).transpose(2, 0, 1, 3))
    d["w2"] = _bf(np.asarray(inputs["W2"], np.float32)
                  .reshape(L, 8, P, H).transpose(2, 0, 1, 3))
    d["mw1"] = np.ascontiguousarray(
        np.asarray(inputs["mW1"], np.float32).reshape(6, P, H)
        .transpose(1, 0, 2))
    d["mw2"] = np.ascontiguousarray(
        np.asarray(inputs["mW2"], np.float32).reshape(2, P, R)
        .transpose(1, 0, 2))
    if nz["bqk"]:
        bq = np.asarray(inputs["bqkv"], np.float32)[:, :H] * scale
        bk = np.asarray(inputs["bqkv"], np.float32)[:, H:2 * H]
        arr = np.zeros((P, L, 4), np.float32)
        for l in range(L):
            arr[:, l, 0] = bq[l, :P]
            arr[:, l, 1] = bq[l, P:]
            arr[:, l, 2] = bk[l, :P]
            arr[:, l, 3] = bk[l, P:]
        d["bqk"] = arr
    if nz["bqv"]:
        d["bqv"] = _bf(np.asarray(inputs["bqkv"], np.float32)[:, 2 * H:])
    if nz["b1"]:
        d["b1"] = np.ascontiguousarray(
            np.asarray(inputs["b1"], np.float32).reshape(L, 8, P)
            .transpose(2, 0, 1))
    if nz["bo"]:
        d["bo"] = _bf(inputs["bo"])
    if nz["b2"]:
        d["b2"] = _bf(inputs["b2"])
    if nz["mb1"]:
        d["mb1"] = np.ascontiguousarray(
            np.asarray(inputs["mb1"], np.float32).reshape(2, P).T)
    if nz["mb2"]:
        d["mb2"] = np.asarray(inputs["mb2"], np.float32).reshape(1, R)
    if nz["ln"]:
        rows = []
        for l in range(L):
            rows += [inputs["ln1_g"][l], inputs["ln1_b"][l],
                     inputs["ln2_g"][l], inputs["ln2_b"][l]]
        rows += [inputs["lnf_g"], inputs["lnf_b"]]
        d["lngb"] = np.asarray(np.stack(rows), np.float32)
    return d


def prepare(inputs):
    """Host-side sharding/prep. Returns (in_maps, assign, KVT, nz)."""
    input_ids = np.asarray(inputs["input_ids"])
    attention_mask = np.asarray(inputs["attention_mask"])
    assert input_ids.shape == (B, S)

    nz = {
        "bqk": np.any(np.asarray(inputs["bqkv"], np.float32)[:, :2 * H] != 0),
        "bqv": np.any(np.asarray(inputs["bqkv"], np.float32)[:, 2 * H:] != 0),
        "bo": np.any(np.asarray(inputs["bo"]) != 0),
        "b1": np.any(np.asarray(inputs["b1"]) != 0),
        "b2": np.any(np.asarray(inputs["b2"]) != 0),
        "mb1": np.any(np.asarray(inputs["mb1"]) != 0),
        "mb2": np.any(np.asarray(inputs["mb2"]) != 0),
        "ln": not all(
            np.all(np.asarray(inputs[k]) == 1) for k in
            ("ln1_g", "ln2_g", "lnf_g")) or not all(
            np.all(np.asarray(inputs[k]) == 0) for k in
            ("ln1_b", "ln2_b", "lnf_b")),
    }

    lengths = attention_mask.astype(np.int64).sum(1)  # [B]
    # active token set per example: valid prefix + any sub/obj gather target
    actives = []
    for ex in range(B):
        act = set(range(int(lengths[ex])))
        c = int(inputs["sub_counts"][ex])
        act |= set(np.asarray(inputs["sub_indices"][ex][:c], np.int64).tolist())
        c = int(inputs["obj_counts"][ex])
        act |= set(np.asarray(inputs["obj_indices"][ex][:c], np.int64).tolist())
        if not act:
            act = {0}
        actives.append(np.array(sorted(act), dtype=np.int64))
    n_act = np.array([len(a) for a in actives])
    # snake-deal sorted examples: slot j on core c gets rank j*8 + (j even? c : 7-c)
    order = np.argsort(-lengths, kind="stable")
    assign = np.empty((NCORES, EPC), dtype=np.int64)
    for j in range(EPC):
        grp = order[j * NCORES:(j + 1) * NCORES]
        assign[:, j] = grp if j % 2 == 0 else grp[::-1]
    KVT = [max(1, int(np.ceil(lengths[assign[:, j]].max() / 128.0)))
           for j in range(EPC)]
    QT = [max(KVT[j], int(np.ceil(n_act[assign[:, j]].max() / 128.0)))
          for j in range(EPC)]
    off = [sum(QT[:e]) for e in range(EPC)]
    TOT = sum(QT)

    shared = _prep_weights(inputs, nz)

    in_maps = []
    for c in range(NCORES):
        exs = assign[c]
        ids_c = np.zeros((TOT * P,), np.int32)
        kb_c = np.full((EPC, 4 * P), -1e9, np.float32)
        wpool_c = np.zeros((EPC, 4 * P, 3), np.float32)
        for j, ex in enumerate(exs):
            A = actives[ex]
            npad = QT[j] * P
            toks = np.zeros((npad,), np.int64)
            toks[:len(A)] = A                      # pad with token position 0
            ids_c[off[j] * P:off[j] * P + npad] = input_ids[ex][toks]
            ln = int(lengths[ex])
            kb_c[j, :ln] = 0.0
            # pool weights in compacted coordinates
            pos = {int(o): i for i, o in enumerate(A)}
            cnt = int(inputs["sub_counts"][ex])
            if cnt > 0:
                idxs = [pos[int(i)] for i in
                        np.asarray(inputs["sub_indices"][ex][:cnt], np.int64)]
                np.add.at(wpool_c[j, :, 0], idxs, 1.0 / cnt)
            cnt = int(inputs["obj_counts"][ex])
            if cnt > 0:
                idxs = [pos[int(i)] for i in
                        np.asarray(inputs["obj_indices"][ex][:cnt], np.int64)]
                np.add.at(wpool_c[j, :, 1], idxs, 1.0 / cnt)
            if ln > 0:
                wpool_c[j, :ln, 2] = 1.0 / ln
            else:
                # reference falls back to plain mean over all S positions;
                # with ln==0 every position is padding - approximate with
                # uniform weights over the compacted set (values identical
                # rows, so result matches mean over duplicates of row 0...
                # cannot happen since lengths >= 1 in practice)
                wpool_c[j, :, 2] = 1.0 / npad
        m = dict(shared)
        m["ids"] = np.ascontiguousarray(ids_c.reshape(TOT, P).T)  # [P, TOT]
        m["kbias"] = np.ascontiguousarray(
            kb_c.reshape(EPC, 4, P).transpose(2, 0, 1))   # [P, EPC, 4]
        m["wpool"] = np.ascontiguousarray(
            wpool_c.reshape(EPC, 4, P, 3).transpose(2, 0, 1, 3))
        in_maps.append(m)
    return in_maps, assign, KVT, QT, nz


def kernel(**inputs):
    in_maps, assign, KVT, QT, nz = prepare(inputs)
    nc = build_program_cached(KVT, QT, nz)
    res = run_bass_kernel_spmd(nc, in_maps, core_ids=list(range(NCORES)))
    logits = np.zeros((B, R), np.float32)
    for c in range(NCORES):
        logits[assign[c]] = res.results[c]["out"]
    return logits


if __name__ == "__main__":
    np.random.seed(0)
    fake = {}
    rng = np.random.default_rng(0)
    fake["input_ids"] = rng.integers(0, V, (B, S)).astype(np.int64)
    ln_ = rng.integers(16, S + 1, (B,))
    fake["attention_mask"] = (np.arange(S)[None] < ln_[:, None]).astype(np.int32)
    fake["sub_indices"] = rng.integers(0, S, (B, KIDX)).astype(np.int64)
    fake["obj_indices"] = rng.integers(0, S, (B, KIDX)).astype(np.int64)
    fake["sub_counts"] = rng.integers(0, KIDX + 1, (B,)).astype(np.int32)
    fake["obj_counts"] = rng.integers(0, KIDX + 1, (B,)).astype(np.int32)
    sc = 0.02
    fake["emb"] = rng.normal(0, sc, (V, H)).astype(np.float32)
    fake["Wqkv"] = rng.normal(0, sc, (L, H, 3 * H)).astype(np.float32)
    fake["bqkv"] = np.zeros((L, 3 * H), np.float32)
    fake["Wo"] = rng.normal(0, sc, (L, H, H)).astype(np.float32)
    fake["bo"] = np.zeros((L, H), np.float32)
    fake["ln1_g"] = np.ones((L, H), np.float32)
    fake["ln1_b"] = np.zeros((L, H), np.float32)
    fake["ln2_g"] = np.ones((L, H), np.float32)
    fake["ln2_b"] = np.zeros((L, H), np.float32)
    fake["W1"] = rng.normal(0, sc, (L, H, FF)).astype(np.float32)
    fake["b1"] = np.zeros((L, FF), np.float32)
    fake["W2"] = rng.normal(0, sc, (L, FF, H)).astype(np.float32)
    fake["b2"] = np.zeros((L, H), np.float32)
    fake["lnf_g"] = np.ones((H,), np.float32)
    fake["lnf_b"] = np.zeros((H,), np.float32)
    fake["mW1"] = rng.normal(0, sc, (3 * H, H)).astype(np.float32)
    fake["mb1"] = np.zeros((H,), np.float32)
    fake["mW2"] = rng.normal(0, sc, (H, R)).astype(np.float32)
    fake["mb2"] = np.zeros((R,), np.float32)
    out = kernel(**fake)
    print(out.shape, out.dtype, np.abs(out).max())

